# revision 24
# baseline (speedup 1.0000x reference)
"""CrossAttention kernel for 8 trn2 NeuronCores.

Reference:
  q = x @ Wq          [n, vq, h]
  k = y @ Wk          [n, vk, h]
  v = y @ Wv          [n, vk, c]
  out = softmax(q k^T / sqrt(h)) @ v        [n, vq, c]
with N=4, VQ=VK=4096, C=128, H=64, fp32.

Sharding: 8 cores = 4 batches x 2 query halves. Each core gets
x-shard [2048, 128] and the full y[n] [4096, 128], computes out-shard
[2048, 128].

Per-core dataflow (all matmuls fp32r = full-rate PE):
  - transpose x,y tiles on PE (exact) -> xT [c, vq], yT [c, vk]
  - qT [h, vq] = Wq^T xT ; kT [h, vk] = Wk^T yT ; v [vk, c] = (yT)^T Wv
  - flash loop over vq tiles of 512:
      scoresT [vk_t=128, vq=512] = kT_tile^T qT_tile     (K=h=64)
      attnT = exp(scale * scoresT)  (ScalarE, PSUM->SBUF, f32r out)
      outT [c, 512] += v_tile^T ... accumulate over 32 vk tiles
      sums [1, 512] += ones^T attnT
  - sums -> DRAM bounce -> [128, 16] per-partition layout, reciprocal
  - out [vq, c] = transpose(outT) * rsum  (PE transpose + DVE scale)
"""

import sys

sys.path.insert(0, "/opt/trn_rl_repo")

from contextlib import ExitStack

import numpy as np

import concourse.bass as bass
import concourse.tile as tile
from concourse import mybir
from concourse.bass_utils import run_bass_kernel_spmd
from concourse.masks import make_identity

F32 = mybir.dt.float32
F32R = mybir.dt.float32r
F16 = mybir.dt.float16
FP8 = mybir.dt.float8e4
P = 128

N, VQ, VK, C, H = 4, 4096, 4096, 128, 64
VQ_PER = VQ // 2          # 2048 queries per core
SCALE = float(H) ** -0.5

# main-loop tiling
VQ_T = 512                # vq tile (psum free dim)
N_VQ_T = VQ_PER // VQ_T   # 4
N_VK_T = VK // P          # 32 vk tiles of 128
CHUNK = 2                 # vk tiles per exp chunk (row-packed pair)


def _split_multi_waits(nc):
    """walrus in this env supports one sync-wait per instruction; hoist
    extras onto same-engine NoOps inserted just before."""
    for fn in nc.m.functions:
        for bb in fn.blocks:
            out = []
            for inst in bb.instructions:
                si = inst.sync_info
                waits = list(si.on_wait) if si and si.on_wait else []
                if len(waits) > 1:
                    for w in waits[:-1]:
                        out.append(mybir.InstNoOp(
                            name=nc.get_next_instruction_name(),
                            engine=inst.engine,
                            ins=[], outs=[],
                            sync_info=mybir.SyncInfo(on_wait=[w], on_update=[]),
                        ))
                    inst.sync_info = mybir.SyncInfo(
                        on_wait=[waits[-1]],
                        on_update=list(si.on_update) if si.on_update else [],
                    )
                out.append(inst)
            bb.instructions = out


def _build():
    nc = bass.Bass()
    x_d = nc.declare_dram_parameter("x", [VQ_PER, C], F32, isOutput=False)
    y_d = nc.declare_dram_parameter("y", [VK, C], F32, isOutput=False)
    wq_d = nc.declare_dram_parameter("Wq", [C, H], F32, isOutput=False)
    wk_d = nc.declare_dram_parameter("Wk", [C, H], F32, isOutput=False)
    wv_d = nc.declare_dram_parameter("Wv", [C, C], F32, isOutput=False)
    o_d = nc.declare_dram_parameter("o", [VQ_PER, C], F32, isOutput=True)

    with tile.TileContext(nc) as tc, ExitStack() as ctx:
        const = ctx.enter_context(tc.tile_pool(name="const", bufs=1))
        persist = ctx.enter_context(tc.tile_pool(name="persist", bufs=1))

        # ---- constants ----
        ident = const.tile([P, P], F32)
        make_identity(nc, ident[:])
        w_stage = const.tile([P, 2 * H + C], F32)
        w_r = const.tile([P, 4 * H + C], F16)

        def stage_weights():
            nc.sync.dma_start(w_stage[:, 0:H], wq_d[:])
            nc.sync.dma_start(w_stage[:, H:2 * H], wk_d[:])
            nc.sync.dma_start(w_stage[:, 2 * H:], wv_d[:])
            nc.vector.tensor_copy(w_r[:, 0:H], w_stage[:, 0:H])
            nc.vector.tensor_copy(w_r[:, H:2 * H], w_stage[:, 0:H])
            nc.vector.tensor_copy(w_r[:, 2 * H:3 * H], w_stage[:, H:2 * H])
            nc.vector.tensor_copy(w_r[:, 3 * H:4 * H], w_stage[:, H:2 * H])
            nc.vector.tensor_copy(w_r[:, 4 * H:], w_stage[:, 2 * H:])
        wqq_r = w_r[:, 0:2 * H]      # [Wq | Wq] -> duplicated qT rows
        wkk_r = w_r[:, 2 * H:4 * H]  # [Wk | Wk] -> duplicated kT rows
        wv_r = w_r[:, 4 * H:]
        ones_f = const.tile([P, 1], F32)
        nc.vector.memset(ones_f[:], 1.0)
        bias_t = const.tile([P, 1], F32)
        nc.vector.memset(bias_t[:], -4.0)
        ones_h = const.tile([P, 1], F16)
        nc.vector.memset(ones_h[:], 1.0)
        ident_h = const.tile([P, P], F16)
        make_identity(nc, ident_h[:])

        # ---- persistent tensors ----
        qT = persist.tile([P, VQ_PER], F16)          # [128, 2048] rows 64:128 dup
        kT = persist.tile([P, VK], F16)              # [128, 4096] rows 64:128 dup
        v_sb = persist.tile([P, N_VK_T, C], F16)    # [128, 32, 128] vk-major
        attnT = persist.tile([P, N_VK_T, VQ_T], F16)  # [128, 32, 512] per vq tile
        sacc = [persist.tile([P, 2 * VQ_T], F16, name=f"sacc{i}")
                for i in range(2)]
        outT = persist.tile([P, VQ_PER], F16)         # [c, 2048]
        out_sb = persist.tile([P, (VQ_PER // P) * C], F32)  # [128, 16*128]
        srow = persist.tile([1, VQ_PER], F32)         # softmax sums, vq-flat

        # ---- phase 1: load + transpose + project ----
        with ExitStack() as pctx:
            ld = pctx.enter_context(tc.tile_pool(name="ld", bufs=12))
            ldh = pctx.enter_context(tc.tile_pool(name="ldh", bufs=3))
            tp_ps = pctx.enter_context(
                tc.tile_pool(name="tp_ps", bufs=2, space="PSUM"))
            pj_ps = pctx.enter_context(
                tc.tile_pool(name="pj_ps", bufs=3, space="PSUM"))
            v_ps = pctx.enter_context(
                tc.tile_pool(name="v_ps", bufs=3, space="PSUM"))
            xyT = pctx.enter_context(tc.tile_pool(name="xyT", bufs=6))

            def issue_loads(src_ap, n_chunks):
                raws = []
                for ch in range(n_chunks):
                    raw = ld.tile([P, 4, P], F32, tag="raw")
                    eng = nc.sync if ch % 2 == 0 else nc.gpsimd
                    eng.dma_start(
                        raw[:],
                        src_ap[ch * 512:(ch + 1) * 512, :]
                        .rearrange("(t p) c -> p t c", p=P),
                    )
                    raws.append(raw)
                return raws

            def process(raw, ch, proj, eng):
                rawh = ldh.tile([P, 4, P], F16, tag="rawh")
                nc.scalar.copy(rawh[:], raw[:])
                t_ps = tp_ps.tile([P, 512], F16, tag="tp")
                for b in range(4):
                    nc.tensor.transpose(
                        t_ps[:, b * P:(b + 1) * P], rawh[:, b, :],
                        ident_h[:])
                t_sb = xyT.tile([P, 512], F16, tag="t_sb")
                nc.vector.tensor_copy(t_sb[:], t_ps[:])
                proj(ch, t_sb)

            def proj_x(ch, xT_sb):
                q_ps = pj_ps.tile([P, 512], F32, tag="qk")
                nc.tensor.matmul(q_ps[:], wqq_r[:], xT_sb[:], start=True, stop=True)
                nc.vector.tensor_copy(qT[:, ch * 512:(ch + 1) * 512], q_ps[:])

            def proj_y(ch, yT_sb):
                k_ps = pj_ps.tile([P, 512], F32, tag="qk")
                nc.tensor.matmul(k_ps[:], wkk_r[:], yT_sb[:], start=True, stop=True)
                nc.vector.tensor_copy(kT[:, ch * 512:(ch + 1) * 512], k_ps[:])
                vp = v_ps.tile([P, 512], F32, tag="vp")
                for b in range(4):
                    nc.tensor.matmul(
                        vp[:, b * P:(b + 1) * P],
                        yT_sb[:, b * P:(b + 1) * P], wv_r[:],
                        start=True, stop=True)
                nc.vector.tensor_copy(
                    v_sb[:, 4 * ch:4 * ch + 4, :], vp[:])

            stage_weights()
            x_raws = issue_loads(x_d, VQ_PER // 512)
            y_raws = issue_loads(y_d, VK // 512)
            for ch, raw in enumerate(x_raws):
                process(raw, ch, proj_x, ch % 2)
            for ch, raw in enumerate(y_raws):
                process(raw, ch, proj_y, ch % 2)

        # ---- phase 2: flash loop over vq tiles ----
        # software-pipelined emission: PE stream = sc(n+1) before pv/sm(n),
        # so the PE never blocks on the activation of the current chunk.
        with ExitStack() as mctx:
            sc_ps = mctx.enter_context(
                tc.tile_pool(name="sc_ps", bufs=2, space="PSUM"))
            pv_ps = mctx.enter_context(
                tc.tile_pool(name="pv_ps", bufs=1, space="PSUM"))
            sm_ps = mctx.enter_context(
                tc.tile_pool(name="sm_ps", bufs=1, space="PSUM"))

            pv_tiles = [None] * N_VQ_T
            starts = list(range(0, N_VK_T, CHUNK))
            work = [(j, s) for j in range(N_VQ_T) for s in starts]

            def emit_scores_exp(j, s):
                sc = sc_ps.tile([P, CHUNK * VQ_T], F32, tag="sc")
                nc.tensor.matmul(
                    sc[:, 0:VQ_T],
                    kT[0:64, s * P:(s + 1) * P],
                    qT[0:64, j * VQ_T:(j + 1) * VQ_T],
                    start=True, stop=True)
                nc.tensor.matmul(
                    sc[:, VQ_T:2 * VQ_T],
                    kT[64:128, (s + 1) * P:(s + 2) * P],
                    qT[64:128, j * VQ_T:(j + 1) * VQ_T],
                    start=True, stop=True, tile_position=(64, 0))
                nc.scalar.activation(
                    attnT[:, s:s + 2, :],
                    sc[:],
                    mybir.ActivationFunctionType.Exp, scale=SCALE,
                    bias=bias_t[:])

            def emit_pv_sm(j, s):
                if s == 0:
                    pva = pv_ps.tile([P, VQ_T], F32, tag="pva", name=f"pva{j}")
                    pvb = pv_ps.tile([P, VQ_T], F32, tag="pvb", name=f"pvb{j}")
                    pv_tiles[j] = (pva, pvb)
                pva, pvb = pv_tiles[j]
                last = (s == starts[-1])
                a_pair = attnT[:, s:s + 2, :]
                for ii in range(CHUNK):
                    i = s + ii
                    # split vk contraction across row groups -> 2x concurrency
                    nc.tensor.matmul(
                        pva[:], v_sb[0:64, i, :], attnT[0:64, i, :],
                        start=(i == 0), stop=(i == N_VK_T - 1))
                    nc.tensor.matmul(
                        pvb[:], v_sb[64:128, i, :], attnT[64:128, i, :],
                        start=(i == 0), stop=(i == N_VK_T - 1),
                        tile_position=(64, 0))
                # softmax denominator partials on DVE (2-byte fast mode)
                sa = sacc[j % 2]
                if s == 0:
                    nc.vector.tensor_copy(sa[:], a_pair)
                else:
                    nc.vector.tensor_tensor(
                        out=sa[:], in0=sa[:], in1=a_pair,
                        op=mybir.AluOpType.add)
                if last:
                    nc.vector.tensor_copy(
                        outT[:, j * VQ_T:(j + 1) * VQ_T], pva[:])
                    nc.vector.tensor_tensor(
                        out=outT[:, j * VQ_T:(j + 1) * VQ_T],
                        in0=outT[:, j * VQ_T:(j + 1) * VQ_T], in1=pvb[:],
                        op=mybir.AluOpType.add)
                    # fold partials across partitions + pair halves on PE
                    sm = sm_ps.tile([1, 2 * VQ_T], F32, tag="sm",
                                    name=f"sm{j}")
                    nc.tensor.matmul(sm[0:1, 0:VQ_T], ones_h[:],
                                     sa[:, 0:VQ_T], start=True, stop=True)
                    nc.tensor.matmul(sm[0:1, VQ_T:2 * VQ_T], ones_h[:],
                                     sa[:, VQ_T:2 * VQ_T],
                                     start=True, stop=True)
                    nc.vector.tensor_copy(
                        srow[:, j * VQ_T:(j + 1) * VQ_T], sm[0:1, 0:VQ_T])
                    nc.vector.tensor_tensor(
                        out=srow[:, j * VQ_T:(j + 1) * VQ_T],
                        in0=srow[:, j * VQ_T:(j + 1) * VQ_T],
                        in1=sm[0:1, VQ_T:2 * VQ_T],
                        op=mybir.AluOpType.add)

            for n, (j, s) in enumerate(work):
                emit_scores_exp(j, s)
                if n > 0:
                    emit_pv_sm(*work[n - 1])
            emit_pv_sm(*work[-1])

        # ---- phase 3: sums transpose + reciprocal + out transpose ----
        with ExitStack() as fctx:
            fin = fctx.enter_context(tc.tile_pool(name="fin", bufs=1))
            f_ps = fctx.enter_context(
                tc.tile_pool(name="f_ps", bufs=4, space="PSUM"))
            s_ps = fctx.enter_context(
                tc.tile_pool(name="s_ps", bufs=1, space="PSUM"))

            n_ot = VQ_PER // P  # 16 output blocks of [128 vq, 128 c]
            # transpose sums row [1, 2048] -> [128, 16] via PE row-transposes
            sT_ps = s_ps.tile([P, n_ot], F32)
            sumsT = fin.tile([P, n_ot], F32)
            rsum = fin.tile([P, n_ot], F32)

            for j in range(n_ot // 4):
                for t in range(4 * j, 4 * j + 4):
                    nc.tensor.transpose(
                        sT_ps[:, t:t + 1], srow[0:1, t * P:(t + 1) * P],
                        ones_f[0:1, 0:1])
                q4 = slice(4 * j, 4 * j + 4)
                nc.vector.tensor_copy(sumsT[:, q4], sT_ps[:, q4])
                nc.vector.reciprocal(rsum[:, q4], sumsT[:, q4])
                for jj in range(4):
                    t = 4 * j + jj
                    o_ps = f_ps.tile([P, P], F16)
                    nc.tensor.transpose(
                        o_ps[:], outT[:, t * P:(t + 1) * P], ident_h[:])
                    nc.vector.tensor_scalar(
                        out=out_sb[:, t * P:(t + 1) * P], in0=o_ps[:],
                        scalar1=rsum[:, t:t + 1], scalar2=None,
                        op0=mybir.AluOpType.mult)
                jsl = slice(j * VQ_T, (j + 1) * VQ_T)
                nc.sync.dma_start(
                    o_d[jsl, :].rearrange("(t p) c -> p t c", p=P),
                    out_sb[:, jsl].rearrange("p (t c) -> p t c", c=C),
                )

    _split_multi_waits(nc)
    return nc


_NC = None


def _get_nc():
    global _NC
    if _NC is None:
        _NC = _build()
    return _NC


def kernel(x, y, Wq, Wk, Wv):
    x = np.ascontiguousarray(x, dtype=np.float32)
    y = np.ascontiguousarray(y, dtype=np.float32)
    Wq = np.ascontiguousarray(Wq, dtype=np.float32)
    Wk = np.ascontiguousarray(Wk, dtype=np.float32)
    Wv = np.ascontiguousarray(Wv, dtype=np.float32)

    nc = _get_nc()
    core_ids = list(range(8))
    in_maps = []
    for core in core_ids:
        n, half = core // 2, core % 2
        in_maps.append({
            "x": x[n, half * VQ_PER:(half + 1) * VQ_PER, :],
            "y": y[n],
            "Wq": Wq, "Wk": Wk, "Wv": Wv,
        })
    res = run_bass_kernel_spmd(nc, in_maps, core_ids)
    out = np.empty((N, VQ, C), dtype=np.float32)
    for core in core_ids:
        n, half = core // 2, core % 2
        out[n, half * VQ_PER:(half + 1) * VQ_PER, :] = res.results[core]["o"]
    return out



# revision 26
# speedup vs baseline: 1.0486x; 1.0486x over previous
"""CrossAttention kernel for 8 trn2 NeuronCores.

Reference:
  q = x @ Wq          [n, vq, h]
  k = y @ Wk          [n, vk, h]
  v = y @ Wv          [n, vk, c]
  out = softmax(q k^T / sqrt(h)) @ v        [n, vq, c]
with N=4, VQ=VK=4096, C=128, H=64, fp32.

Sharding: 8 cores = 4 batches x 2 query halves. Each core gets
x-shard [2048, 128] and the full y[n] [4096, 128], computes out-shard
[2048, 128].

Per-core dataflow (all matmuls fp32r = full-rate PE):
  - transpose x,y tiles on PE (exact) -> xT [c, vq], yT [c, vk]
  - qT [h, vq] = Wq^T xT ; kT [h, vk] = Wk^T yT ; v [vk, c] = (yT)^T Wv
  - flash loop over vq tiles of 512:
      scoresT [vk_t=128, vq=512] = kT_tile^T qT_tile     (K=h=64)
      attnT = exp(scale * scoresT)  (ScalarE, PSUM->SBUF, f32r out)
      outT [c, 512] += v_tile^T ... accumulate over 32 vk tiles
      sums [1, 512] += ones^T attnT
  - sums -> DRAM bounce -> [128, 16] per-partition layout, reciprocal
  - out [vq, c] = transpose(outT) * rsum  (PE transpose + DVE scale)
"""

import sys

sys.path.insert(0, "/opt/trn_rl_repo")

from contextlib import ExitStack

import numpy as np

import concourse.bass as bass
import concourse.tile as tile
from concourse import mybir
from concourse.bass_utils import run_bass_kernel_spmd
from concourse.masks import make_identity

F32 = mybir.dt.float32
F32R = mybir.dt.float32r
F16 = mybir.dt.float16
FP8 = mybir.dt.float8e4
P = 128

N, VQ, VK, C, H = 4, 4096, 4096, 128, 64
VQ_PER = VQ // 2          # 2048 queries per core
SCALE = float(H) ** -0.5

# main-loop tiling
VQ_T = 512                # vq tile (psum free dim)
N_VQ_T = VQ_PER // VQ_T   # 4
N_VK_T = VK // P          # 32 vk tiles of 128
CHUNK = 2                 # vk tiles per exp chunk (row-packed pair)


def _split_multi_waits(nc):
    """walrus in this env supports one sync-wait per instruction; hoist
    extras onto same-engine NoOps inserted just before."""
    for fn in nc.m.functions:
        for bb in fn.blocks:
            out = []
            for inst in bb.instructions:
                si = inst.sync_info
                waits = list(si.on_wait) if si and si.on_wait else []
                if len(waits) > 1:
                    for w in waits[:-1]:
                        out.append(mybir.InstNoOp(
                            name=nc.get_next_instruction_name(),
                            engine=inst.engine,
                            ins=[], outs=[],
                            sync_info=mybir.SyncInfo(on_wait=[w], on_update=[]),
                        ))
                    inst.sync_info = mybir.SyncInfo(
                        on_wait=[waits[-1]],
                        on_update=list(si.on_update) if si.on_update else [],
                    )
                out.append(inst)
            bb.instructions = out


def _build():
    nc = bass.Bass()
    x_d = nc.declare_dram_parameter("x", [VQ_PER, C], F32, isOutput=False)
    y_d = nc.declare_dram_parameter("y", [VK, C], F32, isOutput=False)
    wq_d = nc.declare_dram_parameter("Wq", [C, H], F32, isOutput=False)
    wk_d = nc.declare_dram_parameter("Wk", [C, H], F32, isOutput=False)
    wv_d = nc.declare_dram_parameter("Wv", [C, C], F32, isOutput=False)
    o_d = nc.declare_dram_parameter("o", [VQ_PER, C], F32, isOutput=True)

    with tile.TileContext(nc) as tc, ExitStack() as ctx:
        const = ctx.enter_context(tc.tile_pool(name="const", bufs=1))
        persist = ctx.enter_context(tc.tile_pool(name="persist", bufs=1))

        # ---- constants ----
        ident = const.tile([P, P], F32)
        make_identity(nc, ident[:])
        w_stage = const.tile([P, 2 * H + C], F32)
        w_r = const.tile([P, 4 * H + C], F16)

        def stage_weights():
            nc.sync.dma_start(w_stage[:, 0:H], wq_d[:])
            nc.sync.dma_start(w_stage[:, H:2 * H], wk_d[:])
            nc.sync.dma_start(w_stage[:, 2 * H:], wv_d[:])
            nc.vector.tensor_copy(w_r[:, 0:H], w_stage[:, 0:H])
            nc.vector.tensor_copy(w_r[:, H:2 * H], w_stage[:, 0:H])
            nc.vector.tensor_copy(w_r[:, 2 * H:3 * H], w_stage[:, H:2 * H])
            nc.vector.tensor_copy(w_r[:, 3 * H:4 * H], w_stage[:, H:2 * H])
            nc.vector.tensor_copy(w_r[:, 4 * H:], w_stage[:, 2 * H:])
        wqq_r = w_r[:, 0:2 * H]      # [Wq | Wq] -> duplicated qT rows
        wkk_r = w_r[:, 2 * H:4 * H]  # [Wk | Wk] -> duplicated kT rows
        wv_r = w_r[:, 4 * H:]
        ones_f = const.tile([P, 1], F32)
        nc.vector.memset(ones_f[:], 1.0)
        bias_t = const.tile([P, 1], F32)
        nc.vector.memset(bias_t[:], -4.0)
        ones_h = const.tile([P, 1], F16)
        nc.vector.memset(ones_h[:], 1.0)
        ident_h = const.tile([P, P], F16)
        make_identity(nc, ident_h[:])

        # ---- persistent tensors ----
        qT = persist.tile([P, VQ_PER], F16)          # [128, 2048] rows 64:128 dup
        kT = persist.tile([P, VK], F16)              # [128, 4096] rows 64:128 dup
        v_sb = persist.tile([P, N_VK_T, C], F16)    # [128, 32, 128] vk-major
        attnT = persist.tile([P, N_VK_T, VQ_T], F16)  # [128, 32, 512] per vq tile
        sacc = [persist.tile([P, 2 * VQ_T], F16, name=f"sacc{i}")
                for i in range(2)]
        outT = persist.tile([P, VQ_PER], F16)         # [c, 2048]
        out_sb = persist.tile([P, (VQ_PER // P) * C], F32)  # [128, 16*128]
        srow = persist.tile([1, VQ_PER], F32)         # softmax sums, vq-flat

        # ---- phase 1: load + transpose + project ----
        with ExitStack() as pctx:
            ld = pctx.enter_context(tc.tile_pool(name="ld", bufs=12))
            ldh = pctx.enter_context(tc.tile_pool(name="ldh", bufs=3))
            tp_ps = pctx.enter_context(
                tc.tile_pool(name="tp_ps", bufs=2, space="PSUM"))
            pj_ps = pctx.enter_context(
                tc.tile_pool(name="pj_ps", bufs=3, space="PSUM"))
            v_ps = pctx.enter_context(
                tc.tile_pool(name="v_ps", bufs=3, space="PSUM"))
            xyT = pctx.enter_context(tc.tile_pool(name="xyT", bufs=6))

            def issue_loads(src_ap, n_chunks):
                raws = []
                for ch in range(n_chunks):
                    raw = ld.tile([P, 4, P], F32, tag="raw")
                    nc.sync.dma_start(
                        raw[:],
                        src_ap[ch * 512:(ch + 1) * 512, :]
                        .rearrange("(t p) c -> p t c", p=P),
                    )
                    raws.append(raw)
                return raws

            def process(raw, ch, proj, eng):
                rawh = ldh.tile([P, 4, P], F16, tag="rawh")
                nc.scalar.copy(rawh[:], raw[:])
                t_ps = tp_ps.tile([P, 512], F16, tag="tp")
                for b in range(4):
                    nc.tensor.transpose(
                        t_ps[:, b * P:(b + 1) * P], rawh[:, b, :],
                        ident_h[:])
                t_sb = xyT.tile([P, 512], F16, tag="t_sb")
                nc.vector.tensor_copy(t_sb[:], t_ps[:])
                proj(ch, t_sb)

            def proj_x(ch, xT_sb):
                q_ps = pj_ps.tile([P, 512], F32, tag="qk")
                nc.tensor.matmul(q_ps[:], wqq_r[:], xT_sb[:], start=True, stop=True)
                nc.vector.tensor_copy(qT[:, ch * 512:(ch + 1) * 512], q_ps[:])

            def proj_y(ch, yT_sb):
                k_ps = pj_ps.tile([P, 512], F32, tag="qk")
                nc.tensor.matmul(k_ps[:], wkk_r[:], yT_sb[:], start=True, stop=True)
                nc.vector.tensor_copy(kT[:, ch * 512:(ch + 1) * 512], k_ps[:])
                vp = v_ps.tile([P, 512], F32, tag="vp")
                for b in range(4):
                    nc.tensor.matmul(
                        vp[:, b * P:(b + 1) * P],
                        yT_sb[:, b * P:(b + 1) * P], wv_r[:],
                        start=True, stop=True)
                nc.vector.tensor_copy(
                    v_sb[:, 4 * ch:4 * ch + 4, :], vp[:])

            stage_weights()
            x_raws = issue_loads(x_d, VQ_PER // 512)
            y_raws = issue_loads(y_d, VK // 512)
            for ch, raw in enumerate(x_raws):
                process(raw, ch, proj_x, ch % 2)
            for ch, raw in enumerate(y_raws):
                process(raw, ch, proj_y, ch % 2)

        # ---- phase 2: flash loop over vq tiles ----
        # software-pipelined emission: PE stream = sc(n+1) before pv/sm(n),
        # so the PE never blocks on the activation of the current chunk.
        with ExitStack() as mctx:
            sc_ps = mctx.enter_context(
                tc.tile_pool(name="sc_ps", bufs=2, space="PSUM"))
            pv_ps = mctx.enter_context(
                tc.tile_pool(name="pv_ps", bufs=1, space="PSUM"))
            sm_ps = mctx.enter_context(
                tc.tile_pool(name="sm_ps", bufs=1, space="PSUM"))

            pv_tiles = [None] * N_VQ_T
            starts = list(range(0, N_VK_T, CHUNK))
            work = [(j, s) for j in range(N_VQ_T) for s in starts]

            def emit_scores_exp(j, s):
                sc = sc_ps.tile([P, CHUNK * VQ_T], F32, tag="sc")
                nc.tensor.matmul(
                    sc[:, 0:VQ_T],
                    kT[0:64, s * P:(s + 1) * P],
                    qT[0:64, j * VQ_T:(j + 1) * VQ_T],
                    start=True, stop=True)
                nc.tensor.matmul(
                    sc[:, VQ_T:2 * VQ_T],
                    kT[64:128, (s + 1) * P:(s + 2) * P],
                    qT[64:128, j * VQ_T:(j + 1) * VQ_T],
                    start=True, stop=True, tile_position=(64, 0))
                nc.scalar.activation(
                    attnT[:, s:s + 2, :],
                    sc[:],
                    mybir.ActivationFunctionType.Exp, scale=SCALE,
                    bias=bias_t[:])

            def emit_pv_sm(j, s):
                if s == 0:
                    pva = pv_ps.tile([P, VQ_T], F32, tag="pva", name=f"pva{j}")
                    pvb = pv_ps.tile([P, VQ_T], F32, tag="pvb", name=f"pvb{j}")
                    pv_tiles[j] = (pva, pvb)
                pva, pvb = pv_tiles[j]
                last = (s == starts[-1])
                a_pair = attnT[:, s:s + 2, :]
                for ii in range(CHUNK):
                    i = s + ii
                    # split vk contraction across row groups -> 2x concurrency
                    nc.tensor.matmul(
                        pva[:], v_sb[0:64, i, :], attnT[0:64, i, :],
                        start=(i == 0), stop=(i == N_VK_T - 1))
                    nc.tensor.matmul(
                        pvb[:], v_sb[64:128, i, :], attnT[64:128, i, :],
                        start=(i == 0), stop=(i == N_VK_T - 1),
                        tile_position=(64, 0))
                # softmax denominator partials on DVE (2-byte fast mode)
                sa = sacc[j % 2]
                if s == 0:
                    nc.vector.tensor_copy(sa[:], a_pair)
                else:
                    nc.vector.tensor_tensor(
                        out=sa[:], in0=sa[:], in1=a_pair,
                        op=mybir.AluOpType.add)
                if last:
                    nc.vector.tensor_copy(
                        outT[:, j * VQ_T:(j + 1) * VQ_T], pva[:])
                    nc.vector.tensor_tensor(
                        out=outT[:, j * VQ_T:(j + 1) * VQ_T],
                        in0=outT[:, j * VQ_T:(j + 1) * VQ_T], in1=pvb[:],
                        op=mybir.AluOpType.add)
                    # fold partials across partitions + pair halves on PE
                    sm = sm_ps.tile([1, 2 * VQ_T], F32, tag="sm",
                                    name=f"sm{j}")
                    nc.tensor.matmul(sm[0:1, 0:VQ_T], ones_h[:],
                                     sa[:, 0:VQ_T], start=True, stop=True)
                    nc.tensor.matmul(sm[0:1, VQ_T:2 * VQ_T], ones_h[:],
                                     sa[:, VQ_T:2 * VQ_T],
                                     start=True, stop=True)
                    nc.vector.tensor_copy(
                        srow[:, j * VQ_T:(j + 1) * VQ_T], sm[0:1, 0:VQ_T])
                    nc.vector.tensor_tensor(
                        out=srow[:, j * VQ_T:(j + 1) * VQ_T],
                        in0=srow[:, j * VQ_T:(j + 1) * VQ_T],
                        in1=sm[0:1, VQ_T:2 * VQ_T],
                        op=mybir.AluOpType.add)

            for n, (j, s) in enumerate(work):
                emit_scores_exp(j, s)
                if n > 0:
                    emit_pv_sm(*work[n - 1])
            emit_pv_sm(*work[-1])

        # ---- phase 3: sums transpose + reciprocal + out transpose ----
        with ExitStack() as fctx:
            fin = fctx.enter_context(tc.tile_pool(name="fin", bufs=1))
            f_ps = fctx.enter_context(
                tc.tile_pool(name="f_ps", bufs=6, space="PSUM"))
            s_ps = fctx.enter_context(
                tc.tile_pool(name="s_ps", bufs=1, space="PSUM"))

            n_ot = VQ_PER // P  # 16 output blocks of [128 vq, 128 c]
            # transpose sums row [1, 2048] -> [128, 16] via PE row-transposes
            sT_ps = s_ps.tile([P, n_ot], F32)
            sumsT = fin.tile([P, n_ot], F32)
            rsum = fin.tile([P, n_ot], F32)

            for j in range(n_ot // 4):
                for t in range(4 * j, 4 * j + 4):
                    nc.tensor.transpose(
                        sT_ps[:, t:t + 1], srow[0:1, t * P:(t + 1) * P],
                        ones_f[0:1, 0:1])
                q4 = slice(4 * j, 4 * j + 4)
                nc.vector.tensor_copy(sumsT[:, q4], sT_ps[:, q4])
                nc.vector.reciprocal(rsum[:, q4], sumsT[:, q4])
                for jj in range(4):
                    t = 4 * j + jj
                    o_ps = f_ps.tile([P, P], F16)
                    nc.tensor.transpose(
                        o_ps[:], outT[:, t * P:(t + 1) * P], ident_h[:])
                    nc.vector.tensor_scalar(
                        out=out_sb[:, t * P:(t + 1) * P], in0=o_ps[:],
                        scalar1=rsum[:, t:t + 1], scalar2=None,
                        op0=mybir.AluOpType.mult)
                jsl = slice(j * VQ_T, (j + 1) * VQ_T)
                nc.sync.dma_start(
                    o_d[jsl, :].rearrange("(t p) c -> p t c", p=P),
                    out_sb[:, jsl].rearrange("p (t c) -> p t c", c=C),
                )

    _split_multi_waits(nc)
    return nc


_NC = None


def _get_nc():
    global _NC
    if _NC is None:
        _NC = _build()
    return _NC


def kernel(x, y, Wq, Wk, Wv):
    x = np.ascontiguousarray(x, dtype=np.float32)
    y = np.ascontiguousarray(y, dtype=np.float32)
    Wq = np.ascontiguousarray(Wq, dtype=np.float32)
    Wk = np.ascontiguousarray(Wk, dtype=np.float32)
    Wv = np.ascontiguousarray(Wv, dtype=np.float32)

    nc = _get_nc()
    core_ids = list(range(8))
    in_maps = []
    for core in core_ids:
        n, half = core // 2, core % 2
        in_maps.append({
            "x": x[n, half * VQ_PER:(half + 1) * VQ_PER, :],
            "y": y[n],
            "Wq": Wq, "Wk": Wk, "Wv": Wv,
        })
    res = run_bass_kernel_spmd(nc, in_maps, core_ids)
    out = np.empty((N, VQ, C), dtype=np.float32)
    for core in core_ids:
        n, half = core // 2, core % 2
        out[n, half * VQ_PER:(half + 1) * VQ_PER, :] = res.results[core]["o"]
    return out

